# revision 2
# baseline (speedup 1.0000x reference)
"""Causal self-attention (B=2, T=2048, C=1024, H=16) on 8 TRN2 NeuronCores, v2.

Tensor-parallel over heads: core c owns heads {2c, 2c+1}. All compute bf16.
Host pre-transposes x -> xT [C, B*T] so no on-device x transpose is needed.

Per core:
  qT/kT = w_{q,k}^T @ xT          (^T layout: [128=(2h,64d), B*T])
  v     = xT^T @ w_v               (natural: [t, (2h,64d)] per t-block)
  v_aug = [v | ones] per head      ([128 j, 16*65] tiles)
  per (b, pass, h, jb):  S^T[j, i] = kT_blk^T qT  (i in pass window, i >= j0)
    P^T = exp(S^T) (ACT, psum->sbuf bf16), diag block masked by triu
    O_aug[i, 65] += P^T_blk^T v_aug  (N=65 moving; col 64 = denominator;
                                      256B-bank-safe region layout)
  att[i, (h,d)] = O * (1/den)      (DVE recip + tensor_scalar_mul)
  attT blocks via PE transpose; +v-bias during psum->sbuf copy
  Y[t, e] = attT_blk^T @ ow        (psum -> y_sb bf16 -> DRAM)
Host sums the 8 partial Y's and adds o_b.
"""

import numpy as np
import ml_dtypes

B = 2
T = 2048
C = 1024
H = 16
DH = 64
NCORES = 8
HL = 2                      # heads per core
HCOLS = HL * DH             # 128
KB = C // 128               # 8 contraction blocks
TB = T // 128               # 16 t-blocks per batch
NCH = T // 512              # 4 qkv t-chunks per batch
NPASS = 2                   # i-window passes per batch
PW = T // NPASS             # pass width (1024)
PB = PW // 128              # i-blocks per pass (8)

# oau group offsets (f32 elems): 65-wide groups, none crossing a 2KB bank
OAU_OFF = [65 * g for g in range(7)] + [512]
# f32-elem offsets of two [128,128]-bf16 transpose ping-pong regions, placed
# in the oau tile's second bank after group 7 (which ends at 512+65=577)
TP_OFF = [608, 672]
OAU_W = 736                 # oau tile width in f32 (2944B, fits 2 banks)

_nc_cache = None


def _interleave(primary, filler):
    """Emit primary units with filler units woven in evenly."""
    np_, nf = len(primary), len(filler)
    fi = 0
    for i, u in enumerate(primary):
        u()
        want = int(round((i + 1) * nf / max(np_, 1)))
        while fi < want:
            filler[fi]()
            fi += 1
    while fi < nf:
        filler[fi]()
        fi += 1


def build_bass(dbg=False):
    import concourse.bass as bass
    import concourse.bacc as bacc
    import concourse.tile as tile
    import concourse.mybir as mybir

    F32 = mybir.dt.float32
    BF16 = mybir.dt.bfloat16
    Exp = mybir.ActivationFunctionType.Exp
    Mult = mybir.AluOpType.mult

    nc = bacc.Bacc("TRN2", target_bir_lowering=False, debug=False)

    x_d = nc.dram_tensor("x", [C, B * T], BF16, kind="ExternalInput")   # xT
    w_d = nc.dram_tensor("w", [C, 3 * HCOLS], BF16, kind="ExternalInput")
    bias_d = nc.dram_tensor("bqkv", [HCOLS, 3], F32, kind="ExternalInput")
    ow_d = nc.dram_tensor("ow", [HCOLS, C], BF16, kind="ExternalInput")
    ident_d = nc.dram_tensor("ident", [128, 128], BF16, kind="ExternalInput")
    tri_d = nc.dram_tensor("tri", [128, 128], BF16, kind="ExternalInput")
    y_d = nc.dram_tensor("y", [B * T, C], BF16, kind="ExternalOutput")
    if dbg:
        qT_dbg = nc.dram_tensor("qT_dbg", [128, B * T], BF16,
                                kind="ExternalOutput")
        kT_dbg = nc.dram_tensor("kT_dbg", [128, B * T], BF16,
                                kind="ExternalOutput")
        va_dbg = nc.dram_tensor("va_dbg", [128, TB * 65], BF16,
                                kind="ExternalOutput")
        attT_dbg = nc.dram_tensor("attT_dbg", [128, T], BF16,
                                  kind="ExternalOutput")
        pt_dbg = nc.dram_tensor("pt_dbg", [128, 1024], BF16,
                                kind="ExternalOutput")
        oau_dbg = nc.dram_tensor("oau_dbg", [128, 65], F32,
                                 kind="ExternalOutput")
        attsb_dbg = nc.dram_tensor("attsb_dbg", [128, PB * 128], BF16,
                                   kind="ExternalOutput")

    with tile.TileContext(nc) as tc:
        with (
            tc.tile_pool(name="const", bufs=1) as constp,
            tc.tile_pool(name="xt", bufs=1) as xtp,
            tc.tile_pool(name="qkv", bufs=1) as qkvp,
            tc.tile_pool(name="vaug", bufs=2) as vaugp,
            tc.tile_pool(name="pt", bufs=18) as ptp,
            tc.tile_pool(name="attsb", bufs=2) as attsbp,
            tc.tile_pool(name="attT", bufs=2) as attTp,
            tc.tile_pool(name="nrm", bufs=4) as nrmp,
            tc.tile_pool(name="ysb", bufs=6) as ysbp,
            tc.tile_pool(name="ps", bufs=1, space="PSUM") as ps,
        ):
            # ---- constants ----
            w_sb = constp.tile([128, KB * 3 * 128], BF16)   # [c, (kb, 3*128)]
            ow_sb = constp.tile([128, C], BF16)
            bias_sb = constp.tile([HCOLS, 3], F32)
            ident_sb = constp.tile([128, 128], BF16)
            tri_sb = constp.tile([128, 128], BF16)
            w3 = w_sb[:].rearrange("p (kb d) -> p kb d", d=3 * 128)

            def w_consts():
                w_dv = w_d.rearrange("(kb p) d -> p kb d", p=128)
                for kb in range(KB):  # per-kb so first matmuls start early
                    nc.sync.dma_start(w3[:, kb:kb + 1, :], w_dv[:, kb:kb + 1, :])
                nc.sync.dma_start(bias_sb[:], bias_d[:])

            def late_consts():
                nc.sync.dma_start(ident_sb[:], ident_d[:])
                nc.sync.dma_start(tri_sb[:], tri_d[:])
                nc.sync.dma_start(ow_sb[:], ow_d[:])

            xT = xtp.tile([128, KB * B * T], BF16)          # [c, (kb, bt)]
            xTv = xT[:].rearrange("p (kb t) -> p kb t", t=B * T)
            qT = qkvp.tile([128, B * T], BF16, name="qT")
            kT = qkvp.tile([128, B * T], BF16, name="kT")
            vaug = {}                                        # (b, h) -> tile
            attT = {}                                        # b -> tile

            def xchunk_unit(b, c):
                """DMA one 512-token chunk of xT (all kb blocks)."""
                t0 = b * T + c * 512

                def run():
                    nc.sync.dma_start(
                        xTv[:, :, t0:t0 + 512],
                        x_d.rearrange("(kb p) t -> p kb t", p=128)[
                            :, :, t0:t0 + 512],
                    )
                return run

            def qkv_qk_units(b, m, c):
                """One 512-wide chunk of q^T or k^T as two ~850ns units."""
                t0 = b * T + c * 512
                dstT = (qT, kT)[m]
                state = {}

                def run0():
                    pt_ = ps.tile([128, 512], F32, name="psqk", tag="op",
                                  bufs=2)
                    state["pt"] = pt_
                    for kb in range(4):
                        nc.tensor.matmul(
                            pt_[:], w3[:, kb, m * 128:(m + 1) * 128],
                            xTv[:, kb, t0:t0 + 512],
                            start=(kb == 0), stop=False)

                def run1():
                    pt_ = state["pt"]
                    for kb in range(4, KB):
                        nc.tensor.matmul(
                            pt_[:], w3[:, kb, m * 128:(m + 1) * 128],
                            xTv[:, kb, t0:t0 + 512],
                            start=False, stop=(kb == KB - 1))
                    nc.vector.tensor_scalar_add(
                        dstT[:, t0:t0 + 512], pt_[:], bias_sb[:, m:m + 1])
                return [run0, run1]

            def qkv_v_units(b, c):
                """Natural-layout v for 4 t-blocks (one 512 chunk), both heads,
                as two units (2 t-blocks each)."""
                t0 = b * T + c * 512
                state = {}

                def mk(half):
                    def run():
                        if c == 0 and half == 0:
                            for h in range(HL):
                                va = vaugp.tile([128, TB * 65], BF16,
                                                name=f"vaug{b}{h}", tag=f"va{h}")
                                nc.gpsimd.memset(
                                    va[:].rearrange("p (tb d) -> p tb d", d=65)[
                                        :, :, 64:65], 1.0)
                                vaug[(b, h)] = va
                        if half == 0:
                            vps = ps.tile([128, 512], F32, name="psv",
                                          tag="op", bufs=2)
                            state["vps"] = vps
                        vps = state["vps"]
                        for tbl in (0, 1) if half == 0 else (2, 3):
                            for kb in range(KB):
                                nc.tensor.matmul(
                                    vps[:, tbl * 128:(tbl + 1) * 128],
                                    xTv[:, kb,
                                        t0 + tbl * 128: t0 + tbl * 128 + 128],
                                    w3[:, kb, 2 * 128:3 * 128],
                                    start=(kb == 0), stop=(kb == KB - 1))
                        if half == 1:
                            src = vps[:].rearrange(
                                "p (tb hd) -> p tb hd", hd=128)
                            for h in range(HL):
                                dst = vaug[(b, h)][:].rearrange(
                                    "p (tb d) -> p tb d", d=65)[
                                    :, 4 * c:4 * c + 4, 0:64]
                                nc.vector.tensor_copy(
                                    dst, src[:, :, h * 64:(h + 1) * 64])
                    return run
                return [mk(0), mk(1)]

            def attn_batch_units(b):
                """Full attention for batch b. Per (pass, head): jb-loop with
                score+exp emitted one step ahead of PV; xpose woven into h1.
                Returns one unit list per pass."""
                per_pass = []
                attT[b] = attTp.tile([128, T], BF16, name=f"attT{b}")
                for pa in range(NPASS):
                    att_units = []
                    wstart = pa * PW
                    wjb0 = wstart // 128
                    n_jb = wjb0 + PB
                    att_sb = attsbp.tile([128, PB * 128], BF16,
                                         name=f"attsb{b}{pa}", tag="attsb")
                    for h in range(HL):
                        oau = ps.tile([128, OAU_W], F32, name="psoau",
                                      tag="oau", bufs=1)
                        pts = {}

                        def scoreexp(jb, pa=pa, h=h, pts=pts, wstart=wstart):
                            def run():
                                j0 = jb * 128
                                istart = max(j0, wstart)
                                n = wstart + PW - istart
                                sc = ps.tile([128, 1024], F32, name="pssc",
                                             tag="sc", bufs=2)
                                for s0 in range(0, n, 512):   # moving dim <=512
                                    sw = min(512, n - s0)
                                    nc.tensor.matmul(
                                        sc[:, s0:s0 + sw],
                                        kT[h * 64:(h + 1) * 64,
                                           b * T + j0:b * T + j0 + 128],
                                        qT[h * 64:(h + 1) * 64,
                                           b * T + istart + s0:
                                           b * T + istart + s0 + sw],
                                        start=True, stop=True)
                                pt_ = ptp.tile([128, 1024], BF16, name="pt")
                                nc.scalar.activation(pt_[:, 0:n], sc[:, 0:n],
                                                     Exp)
                                if j0 >= wstart:
                                    nc.vector.tensor_tensor(
                                        pt_[:, 0:128], pt_[:, 0:128],
                                        tri_sb[:], Mult)
                                if dbg and b == 0 and pa == 0 and h == 0 \
                                        and jb == 0:
                                    nc.sync.dma_start(pt_dbg[:], pt_[:])
                                pts[jb] = pt_
                            return run

                        def pv(ibl, pa=pa, h=h, pts=pts, oau=oau,
                               att_sb=att_sb, wstart=wstart, wjb0=wjb0):
                            def run():
                                # full accumulation chain for i-block ibl:
                                # one psum group, opened and closed here
                                i0 = wstart + ibl * 128
                                off = OAU_OFF[ibl]
                                va = vaug[(b, h)]
                                last = wjb0 + ibl
                                for jb in range(last + 1):
                                    istart = max(jb * 128, wstart)
                                    pt_ = pts[jb]
                                    nc.tensor.matmul(
                                        oau[:, off:off + 65],
                                        pt_[:, i0 - istart:i0 - istart + 128],
                                        va[:, jb * 65:jb * 65 + 65],
                                        start=(jb == 0),
                                        stop=(jb == last))
                                if dbg and b == 0 and pa == 0 and h == 0 \
                                        and ibl == 2:
                                    dsb = nrmp.tile([128, 65], F32,
                                                    name="dsb", tag="dsb")
                                    nc.vector.tensor_copy(
                                        dsb[:], oau[:, off:off + 65])
                                    nc.sync.dma_start(oau_dbg[:], dsb[:])
                                rr = nrmp.tile([128, 1], F32, name="rr",
                                               tag="rr")
                                nc.vector.reciprocal(
                                    rr[:], oau[:, off + 64:off + 65])
                                nc.vector.tensor_scalar_mul(
                                    att_sb[:].rearrange(
                                        "p (ib hd) -> p ib hd", hd=128)[
                                        :, ibl, h * 64:(h + 1) * 64],
                                    oau[:, off:off + 64], rr[:, 0:1])
                            return run

                        def xpose(ibl, pa=pa, att_sb=att_sb, oau=oau):
                            def run():
                                off = TP_OFF[ibl & 1]
                                tp = oau[:, off:off + 64].bitcast(BF16)
                                nc.tensor.transpose(
                                    tp,
                                    att_sb[:, ibl * 128:(ibl + 1) * 128],
                                    ident_sb[:])
                                ib = pa * PB + ibl
                                nc.vector.tensor_scalar_add(
                                    attT[b][:, ib * 128:(ib + 1) * 128],
                                    tp, bias_sb[:, 2:3])
                            return run

                        # emission: score/exp stream feeds ibl-major PV chains
                        # (exactly one open psum accumulation group at a time)
                        se = [scoreexp(jb) for jb in range(n_jb)]
                        chains = [pv(ibl) for ibl in range(PB)]
                        units = list(se[:wjb0 + 2])
                        for ibl in range(PB):
                            if wjb0 + ibl + 2 < n_jb:
                                units.append(se[wjb0 + ibl + 2])
                            units.append(chains[ibl])
                            if h == 1:
                                units.append(xpose(ibl))
                        if dbg and b == 0 and pa == 0 and h == 1:
                            def dump_attsb(att_sb=att_sb):
                                nc.sync.dma_start(attsb_dbg[:], att_sb[:])
                            units.append(dump_attsb)
                        att_units.extend(units)
                    per_pass.append(att_units)
                return per_pass

            Copy = mybir.ActivationFunctionType.Copy

            def copy_eng(state, dst, src):
                # psum can only be drained by DVE and ACT; weight 2:1 to DVE
                e = state["cp"] % 3
                state["cp"] += 1
                if e == 2:
                    nc.scalar.activation(dst, src, Copy)
                else:
                    nc.vector.tensor_copy(dst, src)

            def oproj_unit(b, tb, state, tail=False):
                """One t-block of o_proj. Fill mode: 2x N=512 matmuls into the
                "op" psum + rotating copies. Tail mode: 1x N=1024 matmul into
                the (by then idle) "sc" psum + one copy."""
                def run():
                    yo = ysbp.tile([128, C], BF16, name="yo")
                    if tail:
                        op_ps = ps.tile([128, 1024], F32, name="psyt",
                                        tag="sc", bufs=2)
                        for ec in range(2):
                            nc.tensor.matmul(
                                op_ps[:, ec * 512:(ec + 1) * 512],
                                attT[b][:, tb * 128:(tb + 1) * 128],
                                ow_sb[:, ec * 512:(ec + 1) * 512],
                                start=True, stop=True)
                        copy_eng(state, yo[:], op_ps[:])
                    else:
                        for ec in range(2):
                            op_ps = ps.tile([128, 512], F32, name="psy",
                                            tag="op", bufs=2)
                            nc.tensor.matmul(
                                op_ps[:],
                                attT[b][:, tb * 128:(tb + 1) * 128],
                                ow_sb[:, ec * 512:(ec + 1) * 512],
                                start=True, stop=True)
                            copy_eng(state, yo[:, ec * 512:(ec + 1) * 512],
                                     op_ps[:])
                    eng = (nc.sync, nc.scalar)[state["dma"] & 1]
                    state["dma"] += 1
                    eng.dma_start(
                        y_d[b * T + tb * 128:b * T + (tb + 1) * 128, :], yo[:])
                return run

            # ---- schedule ----
            state = {"cp": 0, "dma": 0, "ps": 0}

            def qkv_chunk_units(b, c):
                return (qkv_qk_units(b, 0, c) + qkv_qk_units(b, 1, c)
                        + qkv_v_units(b, c))

            # prefetch x; chunk (0,0) first (longest pole for first matmul),
            # then weights, then the rest
            xchunk_unit(0, 0)()
            w_consts()
            xchunk_unit(0, 1)()
            late_consts()
            xchunk_unit(0, 2)()
            xchunk_unit(0, 3)()
            for c in range(NCH):
                xchunk_unit(1, c)()

            # qkv(0) chunks 0-1 serial: attention pass A only needs these
            for c in (0, 1):
                for u in qkv_chunk_units(0, c):
                    u()

            fill1 = (qkv_chunk_units(0, 2) + qkv_chunk_units(0, 3)
                     + qkv_chunk_units(1, 0) + qkv_chunk_units(1, 1))
            attn0A, attn0B = attn_batch_units(0)
            _interleave(attn0A + attn0B, fill1)

            attn1A, attn1B = attn_batch_units(1)
            oproj0 = [oproj_unit(0, tb, state) for tb in range(TB)]
            oproj1 = [oproj_unit(1, tb, state) for tb in range(TB)]
            fillDA = (qkv_chunk_units(1, 2) + qkv_chunk_units(1, 3)
                      + oproj0[:4])
            _interleave(attn1A, fillDA)
            _interleave(attn1B, oproj0[4:] + oproj1[:10])
            for tb in range(10, TB):
                oproj_unit(1, tb, state, tail=True)()

            if dbg:
                nc.sync.dma_start(qT_dbg[:], qT[:])
                nc.sync.dma_start(kT_dbg[:], kT[:])
                nc.sync.dma_start(va_dbg[:], vaug[(0, 0)][:])
                nc.sync.dma_start(attT_dbg[:], attT[0][:])

    nc.compile()
    return nc


def _prep_inputs(x, qkv_w, qkv_b, o_w):
    """Per-core input maps (head sharding), bf16, host-side transpose of x."""
    bf16 = ml_dtypes.bfloat16
    x = np.asarray(x, dtype=np.float32)
    qkv_w = np.asarray(qkv_w, dtype=np.float32)
    qkv_b = np.asarray(qkv_b, dtype=np.float32)
    o_w = np.asarray(o_w, dtype=np.float32)

    xT = np.ascontiguousarray(
        x.reshape(B * T, C).T).astype(bf16)               # [C, B*T]
    ident = np.eye(128, dtype=np.float32).astype(bf16)
    tri = np.triu(np.ones((128, 128), dtype=np.float32)).astype(bf16)
    s = 1.0 / np.sqrt(DH)

    in_maps = []
    for c in range(NCORES):
        lo = c * HCOLS
        w_c = np.concatenate(
            [qkv_w[:, lo:lo + HCOLS] * s,             # fold 1/sqrt(dh) into q
             qkv_w[:, C + lo:C + lo + HCOLS],
             qkv_w[:, 2 * C + lo:2 * C + lo + HCOLS]], axis=1).astype(bf16)
        b_c = np.stack(
            [qkv_b[lo:lo + HCOLS] * s,
             qkv_b[C + lo:C + lo + HCOLS],
             qkv_b[2 * C + lo:2 * C + lo + HCOLS]], axis=1).astype(np.float32)
        ow_c = o_w[lo:lo + HCOLS, :].astype(bf16)
        in_maps.append({
            "x": xT,
            "w": np.ascontiguousarray(w_c),
            "bqkv": np.ascontiguousarray(b_c),
            "ow": np.ascontiguousarray(ow_c),
            "ident": ident,
            "tri": tri,
        })
    return in_maps


def kernel(x, qkv_w, qkv_b, o_w, o_b):
    global _nc_cache
    from concourse import bass_utils
    if _nc_cache is None:
        _nc_cache = build_bass()
    nc = _nc_cache
    in_maps = _prep_inputs(x, qkv_w, qkv_b, o_w)
    res = bass_utils.run_bass_kernel_spmd(nc, in_maps, core_ids=list(range(NCORES)))
    y = np.zeros((B * T, C), dtype=np.float64)
    for c in range(NCORES):
        y += res.results[c]["y"].astype(np.float64)
    y = (y + np.asarray(o_b, dtype=np.float64)[None, :]).astype(np.float32)
    return y.reshape(B, T, C)
